# revision 1
# baseline (speedup 1.0000x reference)
"""Trainium2 Bass kernel for the tied-weight Critic MLP.

Math (derived from the reference):
  x   = concat(inputs, actions)                  (B, 420), B = 8192
  s   = sum over 30 column-blocks of 14          (B, 14)
  y1  = s @ W1.T + b1                            (B, 512)
  h1  = relu(layernorm_512(y1))        [g1=1, beta1=0, LN over the 30x tile
                                        equals LN over one 512 block]
  y2  = h1 @ (30*W2).T + b2                      (B, 512)
  h2  = relu(layernorm_512(y2))
  V   = h2 @ (30*wV).T + bV                      (B, 1)
  out = tile(V, 30)                              (B, 30)

Sharding: pure data parallelism - batch 8192 split as 1024 rows on each of
8 NeuronCores; weights replicated.

Per-core layout (batch-major, 8 tiles of 128 rows, two groups of 4 tiles):
  All transposes ride the DMA engines in bf16 (PE transposes and fp32
  matmuls are 2-4x slower on the PE): the four s blocks of a group are
  packed 32 partitions apart with a ones column (bias fold) and flipped by
  a single DMA transpose; mm1 runs as 4 bf16 matmuls against a 4x
  replicated weight tile; h1 is emitted in bf16 by the fused
  scale/bias-ReLU activation and flipped by 4 DMA transposes per tile; mm2
  is a K=1 b2-broadcast matmul plus 4 accumulating bf16 matmuls; LayerNorm
  uses bn_stats/bn_aggr; V is a fp32 mul+reduce against broadcast wV.
  Emission is phase-ordered across each group so the engines pipeline.
"""

import numpy as np

N_CORES = 8
B_FULL = 8192
B_CORE = B_FULL // N_CORES  # 1024
P = 128
N_TILES = B_CORE // P  # 8
GROUP = 4  # tiles per phase group
N_AGENTS = 30
IN_F = 14
HID = 512
EPS = 1e-5

_cache = {}


def _build(bV: float, loop_n: int = 1):
    import concourse.bass as bass
    import concourse.tile as tile
    from concourse import bacc, mybir
    from concourse.bass import ts

    f32 = mybir.dt.float32
    bf16 = mybir.dt.bfloat16
    AF = mybir.ActivationFunctionType
    ALU = mybir.AluOpType

    nc = bacc.Bacc("TRN2")

    xin_d = nc.dram_tensor("xin", (B_CORE, 360), f32, kind="ExternalInput")
    xact_d = nc.dram_tensor("xact", (B_CORE, 60), f32, kind="ExternalInput")
    w1t_d = nc.dram_tensor("w1t", (IN_F + 1, HID), bf16, kind="ExternalInput")
    w2t_d = nc.dram_tensor("w2t", (HID, HID), bf16, kind="ExternalInput")
    b2r_d = nc.dram_tensor("b2r", (1, HID), bf16, kind="ExternalInput")
    wvr_d = nc.dram_tensor("wvr", (1, HID), f32, kind="ExternalInput")
    out_d = nc.dram_tensor("out", (B_CORE, N_AGENTS), f32, kind="ExternalOutput")

    def bcast(ap, p=P):
        return bass.AP(tensor=ap.tensor, offset=ap.offset, ap=[[0, p]] + ap.ap[1:])

    with tile.TileContext(nc) as tc:
        with (
            tc.tile_pool(name="singles", bufs=1) as singles,
            tc.tile_pool(name="xp", bufs=2 * GROUP) as xp,
            tc.tile_pool(name="sp", bufs=2) as sp,
            tc.tile_pool(name="hp", bufs=GROUP) as hp,
            tc.tile_pool(name="stat", bufs=2 * GROUP) as stat,
            tc.tile_pool(name="op", bufs=GROUP) as op,
            tc.tile_pool(name="ps_y", bufs=GROUP, space="PSUM") as ps_y,
        ):
            # ---- constants / replicated weights ----
            ones30 = singles.tile([P, N_AGENTS], f32)
            nc.vector.memset(ones30, 1.0)
            ones1 = singles.tile([1, P], bf16)
            nc.vector.memset(ones1, 1.0)
            eps_t = singles.tile([P, 1], f32)
            nc.vector.memset(eps_t, EPS)

            # w1t replicated at partitions 0/32/64/96 (matmul needs lhsT and
            # rhs on the same base partition; the 4 sT slices sit 32 apart)
            w1t = singles.tile([96 + IN_F + 1, HID], bf16)
            for a in range(GROUP):
                nc.sync.dma_start(
                    out=w1t[32 * a : 32 * a + IN_F + 1, :], in_=w1t_d[:, :]
                )
            w2sb = singles.tile([P, 4, HID], bf16)
            nc.sync.dma_start(
                out=w2sb, in_=w2t_d[:, :].rearrange("(c p) n -> p c n", p=P)
            )
            b2r = singles.tile([1, HID], bf16)
            nc.sync.dma_start(out=b2r, in_=b2r_d[:, :])
            wv_bc = singles.tile([P, HID], f32)
            nc.gpsimd.dma_start(out=wv_bc, in_=bcast(wvr_d[:, :]))

            def layer_norm_relu(y_in, h_out):
                st6 = stat.tile([P, 6], f32, tag="st6")
                nc.vector.bn_stats(st6, y_in)
                mv = stat.tile([P, 2], f32, tag="mv")
                nc.vector.bn_aggr(mv, st6)
                rstd = stat.tile([P, 1], f32, tag="rstd")
                nc.scalar.activation(rstd, mv[:, 1:2], AF.Sqrt, bias=eps_t, scale=1.0)
                nc.vector.reciprocal(rstd, rstd)
                # nm = (mean * rstd) * -1
                nm = stat.tile([P, 1], f32, tag="nm")
                nc.vector.tensor_scalar(
                    out=nm,
                    in0=mv[:, 0:1],
                    scalar1=rstd,
                    scalar2=-1.0,
                    op0=ALU.mult,
                    op1=ALU.mult,
                )
                # h = relu(y * rstd + nm) = relu((y - mean) * rstd)
                nc.scalar.activation(h_out, y_in, AF.Relu, bias=nm, scale=rstd)

            def group_body(g):
                t0 = g * GROUP
                # ---- phase A: load x, form s, pack + single DMA transpose ----
                x_ts = []
                for a in range(GROUP):
                    rows = slice((t0 + a) * P, (t0 + a + 1) * P)
                    x_t = xp.tile([P, 420], f32, tag="x")
                    nc.gpsimd.dma_start(out=x_t[:, 0:360], in_=xin_d[rows, :])
                    nc.gpsimd.dma_start(out=x_t[:, 360:420], in_=xact_d[rows, :])
                    x_ts.append(x_t)
                s_f = sp.tile([P, GROUP, IN_F], f32, tag="sf")
                for a in range(GROUP):
                    nc.vector.reduce_sum(
                        s_f[:, a, :],
                        x_ts[a][:, :].rearrange("p (a f) -> p f a", f=IN_F),
                        axis=mybir.AxisListType.X,
                    )
                # s4b columns: tile a at 32a..32a+13, ones column at 32a+14
                # (becomes the bias row of each lhsT slice after transpose)
                s4b = sp.tile([P, P], bf16, tag="s4b")
                nc.vector.memset(s4b, 1.0)
                s4b_v = s4b[:, :].rearrange("p (a q) -> p a q", q=32)
                nc.scalar.copy(out=s4b_v[:, :, 0:IN_F], in_=s_f)
                st4 = sp.tile([P, P], bf16, tag="st4")
                nc.sync.dma_start_transpose(st4, s4b)

                # ---- phase B: mm1 ----
                y1s = []
                for a in range(GROUP):
                    y1 = ps_y.tile([P, HID], f32, tag="y1")
                    nc.tensor.matmul(
                        y1,
                        st4[32 * a : 32 * a + IN_F + 1, :],
                        w1t[32 * a : 32 * a + IN_F + 1, :],
                        start=True,
                        stop=True,
                        tile_position=(32 * a, 0),
                    )
                    y1s.append(y1)

                # ---- phase C: LN1 + ReLU -> bf16 ----
                h1bs = []
                for a in range(GROUP):
                    h1b = hp.tile([P, HID], bf16, tag="h1b")
                    layer_norm_relu(y1s[a], h1b)
                    h1bs.append(h1b)

                # ---- phase D: h1.T via DMA transposes ----
                h1ts = []
                for a in range(GROUP):
                    h1t = hp.tile([P, 4, P], bf16, tag="h1t")
                    for j in range(4):
                        nc.sync.dma_start_transpose(h1t[:, j, :], h1bs[a][:, ts(j, P)])
                    h1ts.append(h1t)

                # ---- phase E: mm2 = b2 (K=1) + 4 accumulating matmuls ----
                y2s = []
                for a in range(GROUP):
                    y2 = ps_y.tile([P, HID], f32, tag="y2")
                    nc.tensor.matmul(y2, ones1, b2r, start=True, stop=False)
                    for j in range(4):
                        nc.tensor.matmul(
                            y2,
                            h1ts[a][:, j, :],
                            w2sb[:, j, :],
                            start=False,
                            stop=(j == 3),
                        )
                    y2s.append(y2)

                # ---- phase F: LN2 + ReLU ----
                h2s = []
                for a in range(GROUP):
                    h2 = hp.tile([P, HID], f32, tag="h2")
                    layer_norm_relu(y2s[a], h2)
                    h2s.append(h2)

                # ---- phase G: V, broadcast to 30 cols, store ----
                for a in range(GROUP):
                    rows = slice((t0 + a) * P, (t0 + a + 1) * P)
                    tmp = hp.tile([P, HID], f32, tag="tmp")
                    v_t = stat.tile([P, 1], f32, tag="v")
                    nc.vector.tensor_mul(tmp, h2s[a], wv_bc)
                    nc.vector.reduce_sum(v_t, tmp, axis=mybir.AxisListType.X)
                    o30 = op.tile([P, N_AGENTS], f32, tag="o30")
                    nc.scalar.activation(o30, ones30, AF.Copy, bias=float(bV), scale=v_t)
                    nc.scalar.dma_start(out=out_d[rows, :], in_=o30)

            def body():
                for g in range(N_TILES // GROUP):
                    group_body(g)

            if loop_n > 1:
                # timing amplification: repeat the identical batch loop_n times
                with tc.For_i(0, loop_n, 1):
                    body()
            else:
                body()

    nc.compile()
    return nc


def _prep(inputs):
    import ml_dtypes

    xin = np.ascontiguousarray(inputs["inputs"], dtype=np.float32)
    xact = np.ascontiguousarray(inputs["actions"], dtype=np.float32)
    w1 = np.asarray(inputs["w1"], np.float32)
    b1 = np.asarray(inputs["b1"], np.float32)
    w2 = np.asarray(inputs["w2"], np.float32)
    b2 = np.asarray(inputs["b2"], np.float32)
    wV = np.asarray(inputs["wV"], np.float32)
    bV = float(np.asarray(inputs["bV"], np.float32).reshape(-1)[0])

    # LN affine params are identity in this model; the kernel folds them away.
    for k, want in (("g1", 1.0), ("g2", 1.0), ("beta1", 0.0), ("beta2", 0.0)):
        if k in inputs:
            assert np.allclose(np.asarray(inputs[k]), want), f"{k} must be {want}"

    bf = ml_dtypes.bfloat16
    w1t = np.ascontiguousarray(
        np.concatenate([w1, b1[:, None]], axis=1).T
    ).astype(bf)  # (15, 512)
    w2t = np.ascontiguousarray((N_AGENTS * w2).T).astype(bf)  # (512, 512)
    b2r = np.ascontiguousarray(b2[None, :]).astype(bf)  # (1, 512)
    wvr = np.ascontiguousarray(N_AGENTS * wV.reshape(1, -1), dtype=np.float32)

    in_maps = []
    for c in range(N_CORES):
        rows = slice(c * B_CORE, (c + 1) * B_CORE)
        in_maps.append(
            {
                "xin": xin[rows],
                "xact": xact[rows],
                "w1t": w1t,
                "w2t": w2t,
                "b2r": b2r,
                "wvr": wvr,
            }
        )
    return in_maps, bV


def _run(inputs, trace=False):
    from concourse.bass_utils import run_bass_kernel_spmd

    in_maps, bV = _prep(inputs)
    if "nc" not in _cache:
        _cache["nc"] = _build(bV)
    res = run_bass_kernel_spmd(
        _cache["nc"], in_maps, core_ids=list(range(N_CORES)), trace=trace
    )
    out = np.concatenate([m["out"] for m in res.results], axis=0)
    return out, res


def kernel(**inputs) -> np.ndarray:
    out, _ = _run(inputs, trace=False)
    return out



# revision 21
# speedup vs baseline: 1.1491x; 1.1491x over previous
"""Trainium2 Bass kernel for the tied-weight Critic MLP.

Math (derived from the reference):
  x   = concat(inputs, actions)                  (B, 420), B = 8192
  s   = sum over 30 column-blocks of 14          (B, 14)
  y1  = s @ W1.T + b1                            (B, 512)
  h1  = relu(layernorm_512(y1))        [g1=1, beta1=0, LN over the 30x tile
                                        equals LN over one 512 block]
  y2  = h1 @ (30*W2).T + b2                      (B, 512)
  h2  = relu(layernorm_512(y2))
  V   = h2 @ (30*wV).T + bV                      (B, 1)
  out = tile(V, 30)                              (B, 30)

Sharding: pure data parallelism - batch 8192 split as 1024 rows on each of
8 NeuronCores; weights replicated. The kernel emits V (B,1); the 30-column
broadcast is done while unsharding on the host.

v2.2 layout (feature-major, transpose-free, measured-cost-balanced):
  * x is host-transposed to (B, 14, 30) so the agent axis is contiguous:
    the DVE reduce runs at unit stride.
  * The 4 s-vectors of a group are packed at 32-partition stride and
    flipped by ONE DMA transpose per group (the v1 kernel's 16 h1
    transposes per group serialized ~58us on the single HWDGE ring).
  * LN1 stats ride the PE: mean1 = wm.s-hat (block-diag, 4 tiles/matmul),
    E[y1^2] = s-hat.T G s-hat with G = What.T What / 512 (15x15).
  * rstd1 > 0 commutes with relu, so it scales the *columns of the mm1
    rhs*; mm1 emits h1 pre-normalized and feature-major -> mm2 needs no
    transposes at all.
  * Layer-2: b2 rides mm2 as a K=1 matmul; PSUM evacuation is split
    between DVE (tensor_scalar + accum -> mean half) and ACT (Copy +
    accum -> other half); y2 centering is an all-bf16 2x tensor_scalar;
    sumsq of the centered y2 is ACT Square + accum; relu and the 30*wV
    product are fused into one scalar_tensor_tensor (max then mult) whose
    accum is V*std2. Per-group (128,4) stat tiles keep the tiny-op count
    amortized over 4 row-tiles.
"""

import numpy as np

N_CORES = 8
B_FULL = 8192
B_CORE = B_FULL // N_CORES  # 1024
P = 128
N_TILES = B_CORE // P  # 8
GROUP = 4  # tiles per phase group (32-partition stride packing)
N_AGENTS = 30
IN_F = 14
K1 = IN_F + 2  # 14 s rows + ones(b1) row + (-mu) row
HID = 512
EPS = 1e-5
SPL = 192  # DVE's share of the y2 PSUM evacuation columns

_cache = {}


def _build(bV: float):
    import concourse.bass as bass
    import concourse.tile as tile
    from concourse import bacc, mybir
    from concourse.bass import ts

    f32 = mybir.dt.float32
    bf16 = mybir.dt.bfloat16
    AF = mybir.ActivationFunctionType
    ALU = mybir.AluOpType

    nc = bacc.Bacc("TRN2")

    xin_d = nc.dram_tensor("xin", (B_CORE, 420), bf16, kind="ExternalInput")
    w1rep_d = nc.dram_tensor("w1rep", (P, HID), bf16, kind="ExternalInput")
    wm14_d = nc.dram_tensor("wm14", (P, GROUP), bf16, kind="ExternalInput")
    wmneg_d = nc.dram_tensor("wmneg", (P, P), bf16, kind="ExternalInput")
    g1bd_d = nc.dram_tensor("g1bd", (P, P), bf16, kind="ExternalInput")
    onesbd_d = nc.dram_tensor("onesbd", (P, GROUP), f32, kind="ExternalInput")
    pbd_d = nc.dram_tensor("pbd", (GROUP, P), f32, kind="ExternalInput")
    w2t_d = nc.dram_tensor("w2t", (HID, HID), bf16, kind="ExternalInput")
    b2r_d = nc.dram_tensor("b2r", (1, HID), bf16, kind="ExternalInput")
    wvr_d = nc.dram_tensor("wvr", (1, HID), bf16, kind="ExternalInput")
    out_d = nc.dram_tensor("out", (B_CORE, 1), f32, kind="ExternalOutput")

    def bcast(ap, p=P):
        return bass.AP(tensor=ap.tensor, offset=ap.offset, ap=[[0, p]] + ap.ap[1:])

    with tile.TileContext(nc) as tc:
        with (
            tc.tile_pool(name="singles", bufs=1) as singles,
            tc.tile_pool(name="sfp", bufs=2 * GROUP) as sfp,
            tc.tile_pool(name="s4p", bufs=2) as s4p,
            tc.tile_pool(name="gstat", bufs=2) as gstat,
            tc.tile_pool(name="hp", bufs=2) as hp,
            tc.tile_pool(name="y2p", bufs=2 * GROUP) as y2p,
            tc.tile_pool(name="junkp", bufs=2) as junkp,
            tc.tile_pool(name="op", bufs=2) as op,
            tc.tile_pool(name="ps1", bufs=2, space="PSUM") as ps1,
            tc.tile_pool(name="ps2", bufs=2, space="PSUM") as ps2,
            tc.tile_pool(name="psg", bufs=2, space="PSUM") as psg,
        ):
            # ---- constants / replicated weights ----
            eps128 = singles.tile([P, 1], f32)
            nc.vector.memset(eps128, EPS)
            ones1 = singles.tile([1, P], bf16)
            nc.vector.memset(ones1, 1.0)

            w1rep = singles.tile([P, HID], bf16)
            nc.gpsimd.dma_start(out=w1rep, in_=w1rep_d[:, :])
            wm14 = singles.tile([P, GROUP], bf16)
            nc.gpsimd.dma_start(out=wm14, in_=wm14_d[:, :])
            wmneg = singles.tile([P, P], bf16)
            nc.gpsimd.dma_start(out=wmneg, in_=wmneg_d[:, :])
            g1bd = singles.tile([P, P], bf16)
            nc.gpsimd.dma_start(out=g1bd, in_=g1bd_d[:, :])
            onesbd = singles.tile([P, GROUP], f32)
            nc.gpsimd.dma_start(out=onesbd, in_=onesbd_d[:, :])
            pbd = singles.tile([GROUP, P], f32)
            nc.gpsimd.dma_start(out=pbd, in_=pbd_d[:, :])
            w2sb = singles.tile([P, 4, HID], bf16)
            nc.sync.dma_start(
                out=w2sb, in_=w2t_d[:, :].rearrange("(c p) n -> p c n", p=P)
            )
            b2r = singles.tile([1, HID], bf16)
            nc.gpsimd.dma_start(out=b2r, in_=b2r_d[:, :])
            wv_bc = singles.tile([P, HID], bf16)
            nc.gpsimd.dma_start(out=wv_bc, in_=bcast(wvr_d[:, :]))

            def group_body(g):
                t0 = g * GROUP
                # ---- A: load x (bf16, agent-axis innermost), DVE agent-sum
                s_fs = []
                for a in range(GROUP):
                    rows = slice((t0 + a) * P, (t0 + a + 1) * P)
                    x_t = sfp.tile([P, 420], bf16, tag="x")
                    nc.gpsimd.dma_start(out=x_t, in_=xin_d[rows, :])
                    s_f = sfp.tile([P, IN_F], f32, tag="sf")
                    nc.vector.tensor_reduce(
                        s_f,
                        x_t[:, :].rearrange("p (f a) -> p f a", a=N_AGENTS),
                        axis=mybir.AxisListType.X,
                        op=ALU.add,
                    )
                    s_fs.append(s_f)

                # ---- B: pack s (bf16) at 32-stride with ones cols; DMA-T ----
                s4b = s4p.tile([P, P], bf16, tag="s4b")
                nc.gpsimd.memset(s4b, 0.0)
                for a in range(GROUP):
                    nc.gpsimd.memset(s4b[:, 32 * a + IN_F : 32 * a + IN_F + 1], 1.0)
                    nc.vector.tensor_copy(s4b[:, 32 * a : 32 * a + IN_F], s_fs[a])
                st4 = s4p.tile([P, P], bf16, tag="st4")
                nc.sync.dma_start_transpose(st4, s4b)

                # ---- C: LN1 stats on the PE (block-diagonal, 4 tiles at once)
                psb = psg.tile([P, 3, P], f32, tag="psb")
                pss = psg.tile([GROUP, 2, P], f32, tag="pss")
                mu4b = psb[:, 0, :]
                gs4 = psb[:, 1, :]
                rstdb = psb[:, 2, :]
                mu4 = pss[:, 0, :]
                ey4 = pss[:, 1, :]
                nc.tensor.matmul(mu4b, wmneg, st4, start=True, stop=True)
                nc.tensor.matmul(mu4, wm14, st4, start=True, stop=True)
                nc.tensor.matmul(gs4, g1bd, st4, start=True, stop=True)
                prod = gstat.tile([P, P], f32, tag="prod")
                nc.vector.tensor_mul(prod, st4, gs4)
                nc.tensor.matmul(ey4, onesbd, prod, start=True, stop=True)
                # merge -mu rows into st4 (st4 rows 32a+15 are 0; mu4b is 0
                # everywhere except rows 32a+15 = -mu)
                st4m = s4p.tile([P, P], bf16, tag="st4m")
                nc.vector.tensor_add(st4m, st4, mu4b)
                musq1 = gstat.tile([GROUP, P], f32, tag="musq1")
                nc.scalar.square(musq1, mu4)
                var1 = gstat.tile([GROUP, P], f32, tag="var1")
                nc.vector.tensor_sub(var1, ey4, musq1)
                std1 = gstat.tile([GROUP, P], f32, tag="std1")
                nc.scalar.activation(
                    std1, var1, AF.Sqrt, bias=eps128[0:GROUP, :], scale=1.0
                )
                rstd4 = gstat.tile([GROUP, P], f32, tag="rstd4")
                nc.vector.reciprocal_approx_fast(rstd4, std1)
                nc.tensor.matmul(rstdb, pbd, rstd4, start=True, stop=True)
                st4s = s4p.tile([P, P], bf16, tag="st4s")
                nc.vector.tensor_mul(st4s, st4m, rstdb)

                # ---- D: mm1 -> normalized-centered y1, feature-major ----
                y1ps = []
                for a in range(GROUP):
                    y1n = ps1.tile([P, HID], f32, tag="y1n")
                    for j in range(4):
                        nc.tensor.matmul(
                            y1n[:, ts(j, P)],
                            w1rep[32 * a : 32 * a + K1, ts(j, P)],
                            st4s[32 * a : 32 * a + K1, :],
                            start=True,
                            stop=True,
                            tile_position=(32 * a, 0),
                        )
                    y1ps.append(y1n)

                # ---- E: relu -> h1 (bf16, feature-major) ----
                h1s = []
                for a in range(GROUP):
                    h1n = hp.tile([P, HID], bf16, tag="h1n")
                    nc.scalar.activation(h1n, y1ps[a], AF.Relu)
                    h1s.append(h1n)

                # ---- F: mm2 = b2 (K=1) + 4 accumulating bf16 matmuls ----
                y2ps = []
                for a in range(GROUP):
                    y2 = ps2.tile([P, HID], f32, tag="y2")
                    nc.tensor.matmul(y2, ones1, b2r, start=True, stop=False)
                    for j in range(4):
                        nc.tensor.matmul(
                            y2,
                            h1s[a][:, ts(j, P)],
                            w2sb[:, j, :],
                            start=False,
                            stop=(j == 3),
                        )
                    y2ps.append(y2)

                # ---- G: evacuate y2 (split DVE/ACT, accum -> mean) ----
                sum2v = gstat.tile([P, GROUP], f32, tag="sum2v")
                sum2s = gstat.tile([P, GROUP], f32, tag="sum2s")
                y2sbs = []
                for a in range(GROUP):
                    y2sb = y2p.tile([P, HID], bf16, tag="y2sb")
                    nc.vector.tensor_scalar(
                        out=y2sb[:, 0:SPL],
                        in0=y2ps[a][:, 0:SPL],
                        scalar1=0.0,
                        scalar2=None,
                        op0=ALU.add,
                        op1=ALU.add,
                        accum_out=sum2v[:, a : a + 1],
                    )
                    nc.scalar.activation(
                        y2sb[:, SPL:HID],
                        y2ps[a][:, SPL:HID],
                        AF.Copy,
                        accum_out=sum2s[:, a : a + 1],
                    )
                    y2sbs.append(y2sb)
                tsum = gstat.tile([P, GROUP], f32, tag="tsum")
                nc.vector.tensor_add(tsum, sum2v, sum2s)
                negmu4 = gstat.tile([P, GROUP], f32, tag="negmu4")
                nc.vector.tensor_scalar(
                    out=negmu4,
                    in0=tsum,
                    scalar1=-1.0 / HID,
                    scalar2=None,
                    op0=ALU.mult,
                )

                # ---- H: center y2 (2x), sumsq (ACT), fused relu*wv (DVE) ----
                ssq4 = gstat.tile([P, GROUP], f32, tag="ssq4")
                vraw4 = gstat.tile([P, GROUP], f32, tag="vraw4")
                for a in range(GROUP):
                    y2c = y2p.tile([P, HID], bf16, tag="y2c")
                    nc.vector.tensor_scalar(
                        out=y2c,
                        in0=y2sbs[a],
                        scalar1=negmu4[:, a : a + 1],
                        scalar2=None,
                        op0=ALU.add,
                    )
                    sqj = junkp.tile([P, HID], bf16, tag="sqj")
                    nc.scalar.activation(
                        sqj, y2c, AF.Square, accum_out=ssq4[:, a : a + 1]
                    )
                    vj = junkp.tile([P, HID], bf16, tag="vj")
                    nc.vector.scalar_tensor_tensor(
                        out=vj,
                        in0=y2c,
                        scalar=0.0,
                        in1=wv_bc,
                        op0=ALU.max,
                        op1=ALU.mult,
                        accum_out=vraw4[:, a : a + 1],
                    )

                # ---- I: v = vraw / sqrt(ssq/512 + eps) + bV; store ----
                std4 = gstat.tile([P, GROUP], f32, tag="std4")
                nc.scalar.activation(
                    std4, ssq4, AF.Sqrt, bias=eps128, scale=1.0 / HID
                )
                rstd4v = gstat.tile([P, GROUP], f32, tag="rstd4v")
                nc.vector.reciprocal_approx_fast(rstd4v, std4)
                v4 = gstat.tile([P, GROUP], f32, tag="v4")
                nc.vector.tensor_mul(v4, vraw4, rstd4v)
                o4 = op.tile([P, GROUP], f32, tag="o4")
                nc.vector.tensor_scalar(
                    out=o4, in0=v4, scalar1=float(bV), scalar2=None, op0=ALU.add
                )
                rows = slice(t0 * P, (t0 + GROUP) * P)
                nc.sync.dma_start(
                    out=out_d[rows, :].rearrange("(a p) c -> p (a c)", p=P),
                    in_=o4,
                )

            for g in range(N_TILES // GROUP):
                group_body(g)

    nc.compile()
    return nc


def _prep(inputs):
    import ml_dtypes

    bf = ml_dtypes.bfloat16

    xin = np.concatenate(
        [
            np.asarray(inputs["inputs"], np.float32),
            np.asarray(inputs["actions"], np.float32),
        ],
        axis=1,
    )  # (8192, 420)
    # agent axis innermost for a unit-stride DVE reduce
    xin = np.ascontiguousarray(
        xin.reshape(B_FULL, N_AGENTS, IN_F).transpose(0, 2, 1).reshape(B_FULL, 420)
    ).astype(bf)
    w1 = np.asarray(inputs["w1"], np.float32)  # (512, 14)
    b1 = np.asarray(inputs["b1"], np.float32)  # (512,)
    w2 = np.asarray(inputs["w2"], np.float32)  # (512, 512)
    b2 = np.asarray(inputs["b2"], np.float32)
    wV = np.asarray(inputs["wV"], np.float32)
    bV = float(np.asarray(inputs["bV"], np.float32).reshape(-1)[0])

    # LN affine params are identity in this model; the kernel folds them away.
    for k, want in (("g1", 1.0), ("g2", 1.0), ("beta1", 0.0), ("beta2", 0.0)):
        if k in inputs:
            assert np.allclose(np.asarray(inputs[k]), want), f"{k} must be {want}"

    what = np.concatenate([w1, b1[:, None]], axis=1)  # (512, 15)
    wm1 = what.mean(axis=0)  # (15,)
    G1 = (what.T @ what) / HID  # (15, 15)

    # lhsT for mm1: rows 0..13 = W1.T, row 14 = b1, row 15 = ones (-mu slot);
    # replicated at partition offsets 0/32/64/96.
    w1p = np.concatenate([what.T, np.ones((1, HID), np.float32)], axis=0)  # (16,512)
    w1rep = np.zeros((P, HID), np.float32)
    wm14 = np.zeros((P, GROUP), np.float32)
    wmneg = np.zeros((P, P), np.float32)
    g1bd = np.zeros((P, P), np.float32)
    onesbd = np.zeros((P, GROUP), np.float32)
    pbd = np.zeros((GROUP, P), np.float32)
    for a in range(GROUP):
        o = 32 * a
        w1rep[o : o + K1, :] = w1p
        wm14[o : o + IN_F + 1, a] = wm1
        wmneg[o : o + IN_F + 1, o + IN_F + 1] = -wm1
        g1bd[o : o + IN_F + 1, o : o + IN_F + 1] = G1
        onesbd[o : o + IN_F + 1, a] = 1.0
        pbd[a, o : o + K1] = 1.0

    w2t = np.ascontiguousarray((N_AGENTS * w2).T).astype(bf)  # (512, 512)
    b2r = np.ascontiguousarray(b2[None, :]).astype(bf)  # (1, 512)
    wvr = np.ascontiguousarray(N_AGENTS * wV.reshape(1, -1)).astype(bf)

    common = {
        "w1rep": w1rep.astype(bf),
        "wm14": wm14.astype(bf),
        "wmneg": wmneg.astype(bf),
        "g1bd": g1bd.astype(bf),
        "onesbd": onesbd,
        "pbd": pbd,
        "w2t": w2t,
        "b2r": b2r,
        "wvr": wvr,
    }
    in_maps = []
    for c in range(N_CORES):
        rows = slice(c * B_CORE, (c + 1) * B_CORE)
        in_maps.append({"xin": xin[rows], **common})
    return in_maps, bV


def _run(inputs, trace=False):
    from concourse.bass_utils import run_bass_kernel_spmd

    in_maps, bV = _prep(inputs)
    if "nc" not in _cache:
        _cache["nc"] = _build(bV)
    res = run_bass_kernel_spmd(
        _cache["nc"], in_maps, core_ids=list(range(N_CORES)), trace=trace
    )
    v = np.concatenate([m["out"] for m in res.results], axis=0)  # (8192, 1)
    out = np.ascontiguousarray(np.tile(v, (1, N_AGENTS)))  # broadcast = unshard
    return out, res


def kernel(**inputs) -> np.ndarray:
    out, _ = _run(inputs, trace=False)
    return out


# revision 22
# speedup vs baseline: 1.6171x; 1.4072x over previous
"""Trainium2 Bass kernel for the tied-weight Critic MLP.

Math (derived from the reference):
  x   = concat(inputs, actions)                  (B, 420), B = 8192
  s   = sum over 30 column-blocks of 14          (B, 14)
  y1  = s @ W1.T + b1                            (B, 512)
  h1  = relu(layernorm_512(y1))        [g1=1, beta1=0, LN over the 30x tile
                                        equals LN over one 512 block]
  y2  = h1 @ (30*W2).T + b2                      (B, 512)
  h2  = relu(layernorm_512(y2))
  V   = h2 @ (30*wV).T + bV                      (B, 1)
  out = tile(V, 30)                              (B, 30)

Sharding: pure data parallelism - batch 8192 split as 1024 rows on each of
8 NeuronCores; weights replicated. The kernel emits V (B,1); the 30-column
broadcast is done while unsharding on the host.

v2.3 layout (feature-major, transpose-free, analytically-centered):
  * x is host-transposed to (B, 14, 30): the agent-sum reduce is unit
    stride; it writes bf16 directly into the packed s4b tile.
  * One DMA transpose per 4-tile group flips the packed s vectors (vs 16
    h1 DMA transposes per group in v1, which serialized ~58us on the
    single HWDGE ring).
  * LN1 stats ride the PE via block-diagonal host matrices: mean1 =
    wm.s-hat, E[y1^2] = s-hat.T G s-hat (G = What.T What / 512, 15x15).
  * rstd1 > 0 commutes with relu, so it scales the mm1 rhs columns; mm1
    emits h1 pre-normalized and feature-major -> mm2 needs no transposes.
  * Layer 2 is centered for free: mm2 uses W2c = 30W2.T - rowmean and
    b2c = b2 - mean(b2), so mean_g(y2c) == 0 by construction. LN2 then
    needs only sumsq (one DVE op with accum); relu and the 30*wV product
    fuse into one scalar_tensor_tensor (max, mult, accum); rstd2 is
    applied to the accumulated scalar V at the end (relu commutes with
    the positive per-row scale).
"""

import numpy as np

N_CORES = 8
B_FULL = 8192
B_CORE = B_FULL // N_CORES  # 1024
P = 128
N_TILES = B_CORE // P  # 8
GROUP = 4  # tiles per phase group (32-partition stride packing)
N_AGENTS = 30
IN_F = 14
K1 = IN_F + 2  # 14 s rows + ones(b1) row + (-mu) row
HID = 512
EPS = 1e-5

_cache = {}


def _build(bV: float):
    import concourse.bass as bass
    import concourse.tile as tile
    from concourse import bacc, mybir
    from concourse.bass import ts

    f32 = mybir.dt.float32
    bf16 = mybir.dt.bfloat16
    AF = mybir.ActivationFunctionType
    ALU = mybir.AluOpType

    nc = bacc.Bacc("TRN2")

    xin_d = nc.dram_tensor("xin", (B_CORE, 420), bf16, kind="ExternalInput")
    w1rep_d = nc.dram_tensor("w1rep", (P, HID), bf16, kind="ExternalInput")
    wm14_d = nc.dram_tensor("wm14", (P, GROUP), bf16, kind="ExternalInput")
    wmneg_d = nc.dram_tensor("wmneg", (P, P), bf16, kind="ExternalInput")
    g1bd_d = nc.dram_tensor("g1bd", (P, P), bf16, kind="ExternalInput")
    onesbd_d = nc.dram_tensor("onesbd", (P, GROUP), f32, kind="ExternalInput")
    pbd_d = nc.dram_tensor("pbd", (GROUP, P), f32, kind="ExternalInput")
    w2c_d = nc.dram_tensor("w2c", (HID, HID), bf16, kind="ExternalInput")
    b2c_d = nc.dram_tensor("b2c", (1, HID), bf16, kind="ExternalInput")
    wvr_d = nc.dram_tensor("wvr", (1, HID), bf16, kind="ExternalInput")
    out_d = nc.dram_tensor("out", (B_CORE, 1), f32, kind="ExternalOutput")

    def bcast(ap, p=P):
        return bass.AP(tensor=ap.tensor, offset=ap.offset, ap=[[0, p]] + ap.ap[1:])

    with tile.TileContext(nc) as tc:
        with (
            tc.tile_pool(name="singles", bufs=1) as singles,
            tc.tile_pool(name="sfp", bufs=2 * GROUP) as sfp,
            tc.tile_pool(name="s4p", bufs=2) as s4p,
            tc.tile_pool(name="gstat", bufs=2) as gstat,
            tc.tile_pool(name="hp", bufs=2) as hp,
            tc.tile_pool(name="y2p", bufs=2 * GROUP) as y2p,
            tc.tile_pool(name="junkp", bufs=2) as junkp,
            tc.tile_pool(name="op", bufs=2) as op,
            tc.tile_pool(name="ps1", bufs=2, space="PSUM") as ps1,
            tc.tile_pool(name="ps2", bufs=2, space="PSUM") as ps2,
            tc.tile_pool(name="psg", bufs=2, space="PSUM") as psg,
        ):
            # ---- constants / replicated weights ----
            eps128 = singles.tile([P, 1], f32)
            nc.vector.memset(eps128, EPS)
            ones1 = singles.tile([1, P], bf16)
            nc.vector.memset(ones1, 1.0)

            w1rep = singles.tile([P, HID], bf16)
            nc.gpsimd.dma_start(out=w1rep, in_=w1rep_d[:, :])
            wm14 = singles.tile([P, GROUP], bf16)
            nc.gpsimd.dma_start(out=wm14, in_=wm14_d[:, :])
            wmneg = singles.tile([P, P], bf16)
            nc.gpsimd.dma_start(out=wmneg, in_=wmneg_d[:, :])
            g1bd = singles.tile([P, P], bf16)
            nc.gpsimd.dma_start(out=g1bd, in_=g1bd_d[:, :])
            onesbd = singles.tile([P, GROUP], f32)
            nc.gpsimd.dma_start(out=onesbd, in_=onesbd_d[:, :])
            pbd = singles.tile([GROUP, P], f32)
            nc.gpsimd.dma_start(out=pbd, in_=pbd_d[:, :])
            w2sb = singles.tile([P, 4, HID], bf16)
            nc.sync.dma_start(
                out=w2sb, in_=w2c_d[:, :].rearrange("(c p) n -> p c n", p=P)
            )
            b2c = singles.tile([1, HID], bf16)
            nc.gpsimd.dma_start(out=b2c, in_=b2c_d[:, :])
            wv_bc = singles.tile([P, HID], bf16)
            nc.gpsimd.dma_start(out=wv_bc, in_=bcast(wvr_d[:, :]))

            def group_body(g):
                t0 = g * GROUP
                # ---- A: load x; unit-stride agent-sum straight into s4b ----
                s4b = s4p.tile([P, P], bf16, tag="s4b")
                nc.gpsimd.memset(s4b, 0.0)
                x_ts = []
                for a in range(GROUP):
                    rows = slice((t0 + a) * P, (t0 + a + 1) * P)
                    x_t = sfp.tile([P, 420], bf16, tag="x")
                    eng = nc.sync if a % 2 == 0 else nc.gpsimd
                    eng.dma_start(out=x_t, in_=xin_d[rows, :])
                    x_ts.append(x_t)
                for a in range(GROUP):
                    nc.gpsimd.memset(s4b[:, 32 * a + IN_F : 32 * a + IN_F + 1], 1.0)
                    with nc.allow_low_precision("s accumulates 30 bf16 terms"):
                        nc.vector.tensor_reduce(
                            s4b[:, 32 * a : 32 * a + IN_F],
                            x_ts[a][:, :].rearrange("p (f a) -> p f a", a=N_AGENTS),
                            axis=mybir.AxisListType.X,
                            op=ALU.add,
                        )
                st4 = s4p.tile([P, P], bf16, tag="st4")
                nc.sync.dma_start_transpose(st4, s4b)

                # ---- B: LN1 stats on the PE (block-diagonal, 4 tiles/matmul)
                psb = psg.tile([P, 3, P], f32, tag="psb")
                pss = psg.tile([GROUP, 2, P], f32, tag="pss")
                mu4b = psb[:, 0, :]
                gs4 = psb[:, 1, :]
                rstdb = psb[:, 2, :]
                mu4 = pss[:, 0, :]
                ey4 = pss[:, 1, :]
                nc.tensor.matmul(mu4b, wmneg, st4, start=True, stop=True)
                nc.tensor.matmul(mu4, wm14, st4, start=True, stop=True)
                nc.tensor.matmul(gs4, g1bd, st4, start=True, stop=True)
                prod = gstat.tile([P, P], f32, tag="prod")
                nc.vector.tensor_mul(prod, st4, gs4)
                nc.tensor.matmul(ey4, onesbd, prod, start=True, stop=True)
                # merge -mu rows into st4 (st4 rows 32a+15 are 0; mu4b is 0
                # everywhere except rows 32a+15 = -mu)
                st4m = s4p.tile([P, P], bf16, tag="st4m")
                nc.vector.tensor_add(st4m, st4, mu4b)
                musq1 = gstat.tile([GROUP, P], f32, tag="musq1")
                nc.scalar.square(musq1, mu4)
                var1 = gstat.tile([GROUP, P], f32, tag="var1")
                nc.vector.tensor_sub(var1, ey4, musq1)
                std1 = gstat.tile([GROUP, P], f32, tag="std1")
                nc.scalar.activation(
                    std1, var1, AF.Sqrt, bias=eps128[0:GROUP, :], scale=1.0
                )
                rstd4 = gstat.tile([GROUP, P], f32, tag="rstd4")
                nc.vector.reciprocal_approx_fast(rstd4, std1)
                nc.tensor.matmul(rstdb, pbd, rstd4, start=True, stop=True)
                st4s = s4p.tile([P, P], bf16, tag="st4s")
                nc.vector.tensor_mul(st4s, st4m, rstdb)

                # ---- C: mm1 -> normalized-centered y1, feature-major ----
                y1ps = []
                for a in range(GROUP):
                    y1n = ps1.tile([P, HID], f32, tag="y1n")
                    for j in range(4):
                        nc.tensor.matmul(
                            y1n[:, ts(j, P)],
                            w1rep[32 * a : 32 * a + K1, ts(j, P)],
                            st4s[32 * a : 32 * a + K1, :],
                            start=True,
                            stop=True,
                            tile_position=(32 * a, 0),
                        )
                    y1ps.append(y1n)

                # ---- D: relu -> h1 (bf16, feature-major) ----
                h1s = []
                for a in range(GROUP):
                    h1n = hp.tile([P, HID], bf16, tag="h1n")
                    nc.scalar.activation(h1n, y1ps[a], AF.Relu)
                    h1s.append(h1n)

                # ---- E: mm2 (centered weights): y2c straight off the PE ----
                y2ps = []
                for a in range(GROUP):
                    y2 = ps2.tile([P, HID], f32, tag="y2")
                    nc.tensor.matmul(y2, ones1, b2c, start=True, stop=False)
                    for j in range(4):
                        nc.tensor.matmul(
                            y2,
                            h1s[a][:, ts(j, P)],
                            w2sb[:, j, :],
                            start=False,
                            stop=(j == 3),
                        )
                    y2ps.append(y2)

                # ---- F: evacuate (ACT), sumsq + fused relu*wv (DVE) ----
                ssq4 = gstat.tile([P, GROUP], f32, tag="ssq4")
                vraw4 = gstat.tile([P, GROUP], f32, tag="vraw4")
                for a in range(GROUP):
                    y2cb = y2p.tile([P, HID], bf16, tag="y2cb")
                    nc.scalar.copy(out=y2cb, in_=y2ps[a])
                    sqj = junkp.tile([P, HID], bf16, tag="sqj")
                    nc.vector.scalar_tensor_tensor(
                        out=sqj,
                        in0=y2cb,
                        scalar=0.0,
                        in1=y2cb,
                        op0=ALU.bypass,
                        op1=ALU.mult,
                        accum_out=ssq4[:, a : a + 1],
                    )
                    vj = junkp.tile([P, HID], bf16, tag="vj")
                    nc.vector.scalar_tensor_tensor(
                        out=vj,
                        in0=y2cb,
                        scalar=0.0,
                        in1=wv_bc,
                        op0=ALU.max,
                        op1=ALU.mult,
                        accum_out=vraw4[:, a : a + 1],
                    )

                # ---- G: v = vraw / sqrt(ssq/512 + eps) + bV; store ----
                std4 = gstat.tile([P, GROUP], f32, tag="std4")
                nc.scalar.activation(
                    std4, ssq4, AF.Sqrt, bias=eps128, scale=1.0 / HID
                )
                rstd4v = gstat.tile([P, GROUP], f32, tag="rstd4v")
                nc.vector.reciprocal_approx_fast(rstd4v, std4)
                v4 = gstat.tile([P, GROUP], f32, tag="v4")
                nc.vector.tensor_mul(v4, vraw4, rstd4v)
                o4 = op.tile([P, GROUP], f32, tag="o4")
                nc.vector.tensor_scalar(
                    out=o4, in0=v4, scalar1=float(bV), scalar2=None, op0=ALU.add
                )
                rows = slice(t0 * P, (t0 + GROUP) * P)
                nc.sync.dma_start(
                    out=out_d[rows, :].rearrange("(a p) c -> p (a c)", p=P),
                    in_=o4,
                )

            for g in range(N_TILES // GROUP):
                group_body(g)

    nc.compile()
    return nc


def _prep(inputs):
    import ml_dtypes

    bf = ml_dtypes.bfloat16

    xin = np.concatenate(
        [
            np.asarray(inputs["inputs"], np.float32),
            np.asarray(inputs["actions"], np.float32),
        ],
        axis=1,
    )  # (8192, 420)
    # agent axis innermost for a unit-stride DVE reduce
    xin = np.ascontiguousarray(
        xin.reshape(B_FULL, N_AGENTS, IN_F).transpose(0, 2, 1).reshape(B_FULL, 420)
    ).astype(bf)
    w1 = np.asarray(inputs["w1"], np.float32)  # (512, 14)
    b1 = np.asarray(inputs["b1"], np.float32)  # (512,)
    w2 = np.asarray(inputs["w2"], np.float32)  # (512, 512)
    b2 = np.asarray(inputs["b2"], np.float32)
    wV = np.asarray(inputs["wV"], np.float32)
    bV = float(np.asarray(inputs["bV"], np.float32).reshape(-1)[0])

    # LN affine params are identity in this model; the kernel folds them away.
    for k, want in (("g1", 1.0), ("g2", 1.0), ("beta1", 0.0), ("beta2", 0.0)):
        if k in inputs:
            assert np.allclose(np.asarray(inputs[k]), want), f"{k} must be {want}"

    what = np.concatenate([w1, b1[:, None]], axis=1)  # (512, 15)
    wm1 = what.mean(axis=0)  # (15,)
    G1 = (what.T @ what) / HID  # (15, 15)

    # lhsT for mm1: rows 0..13 = W1.T, row 14 = b1, row 15 = ones (-mu slot);
    # replicated at partition offsets 0/32/64/96.
    w1p = np.concatenate([what.T, np.ones((1, HID), np.float32)], axis=0)  # (16,512)
    w1rep = np.zeros((P, HID), np.float32)
    wm14 = np.zeros((P, GROUP), np.float32)
    wmneg = np.zeros((P, P), np.float32)
    g1bd = np.zeros((P, P), np.float32)
    onesbd = np.zeros((P, GROUP), np.float32)
    pbd = np.zeros((GROUP, P), np.float32)
    for a in range(GROUP):
        o = 32 * a
        w1rep[o : o + K1, :] = w1p
        wm14[o : o + IN_F + 1, a] = wm1
        wmneg[o : o + IN_F + 1, o + IN_F + 1] = -wm1
        g1bd[o : o + IN_F + 1, o : o + IN_F + 1] = G1
        onesbd[o : o + IN_F + 1, a] = 1.0
        pbd[a, o : o + K1] = 1.0

    # layer-2 analytic centering: y2c = h1 @ W2c + b2c has zero g-mean
    w2t = (N_AGENTS * w2).T.astype(np.float32)  # (512f, 512g)
    w2c = w2t - w2t.mean(axis=1, keepdims=True)
    b2c = (b2 - b2.mean())[None, :]

    common = {
        "w1rep": w1rep.astype(bf),
        "wm14": wm14.astype(bf),
        "wmneg": wmneg.astype(bf),
        "g1bd": g1bd.astype(bf),
        "onesbd": onesbd,
        "pbd": pbd,
        "w2c": np.ascontiguousarray(w2c).astype(bf),
        "b2c": np.ascontiguousarray(b2c).astype(bf),
        "wvr": np.ascontiguousarray(N_AGENTS * wV.reshape(1, -1)).astype(bf),
    }
    in_maps = []
    for c in range(N_CORES):
        rows = slice(c * B_CORE, (c + 1) * B_CORE)
        in_maps.append({"xin": xin[rows], **common})
    return in_maps, bV


def _run(inputs, trace=False):
    from concourse.bass_utils import run_bass_kernel_spmd

    in_maps, bV = _prep(inputs)
    if "nc" not in _cache:
        _cache["nc"] = _build(bV)
    res = run_bass_kernel_spmd(
        _cache["nc"], in_maps, core_ids=list(range(N_CORES)), trace=trace
    )
    v = np.concatenate([m["out"] for m in res.results], axis=0)  # (8192, 1)
    out = np.ascontiguousarray(np.tile(v, (1, N_AGENTS)))  # broadcast = unshard
    return out, res


def kernel(**inputs) -> np.ndarray:
    out, _ = _run(inputs, trace=False)
    return out
